# revision 6
# baseline (speedup 1.0000x reference)
"""ComplexRNN Trainium2 kernel.

10-layer tanh RNN, B=1024, T=512, D=16, H=30, final FC on last timestep.

Strategy (per core, 8-way batch-parallel, 128 batch rows/core):
  - Hidden-major layout: state h^l lives in SBUF as [30 partitions, 128 batch].
  - Layer wavefront: at step s, layer l computes timestep t = s - l. All
    10 layers advance each step; all dependencies are on step s-1.
  - States packed into 3 "region" windows of 128 partitions (4 slots of 32):
      R0 = [h0 h1 h2 h3], R1 = [h3' h4 h5 h6], R2 = [h6' h7 h8 h9]
    (h3', h6' are duplicates written by extra matmuls so each layer finds
    its feed + recurrent state inside one 128-partition window).
  - Each layer = ONE fp16 matmul (K=128 window, M=30/31, N=128, col-tiled)
    with W_ih^T / W_hh^T rows placed at the slot positions and the combined
    bias on a "ones row" (partition 126 of each window). The ones row is
    self-regenerating: slot-3 matmuls carry an extra output column that maps
    ones -> 30.0 -> tanh -> 1.0.
  - Layer 0's input contribution x_t @ W_ih0^T comes from an "inject" matmul
    reading a pre-transposed x ring buffer (host prepares XT[g, 16*(t%8)+d, b]).
  - tanh: 2 ACTIVATEs per step (PSUM->SBUF): [128,256] for R0+R1, [128,128]
    for R2 -- ScalarE is the bottleneck engine, everything else overlaps.
"""

import copy
import numpy as np

import concourse.bass as bass
import concourse.tile as tile
from concourse import mybir
from concourse import bass_utils

N_CORES = 8
B, T, D, H, L = 1024, 512, 16, 30, 10
BC = B // N_CORES          # batch per core = 128
NBLK = (T + 7) // 8        # 64 x-blocks of 8 timesteps
XT_BLOCKS = NBLK + 8       # pad so ring prefetch never goes OOB
RING = 8                   # resident x blocks

F16 = mybir.dt.float16
F32 = mybir.dt.float32

# weight buffer column layout (each slot 32 cols)
W_L0REC = 0
W_L = lambda l: l          # l = 1..9 -> cols l*32
W_INJ = lambda v: 10 + v   # v = 0..7
W_FC = 18
W_SLOTS = 19


def _split_sync_waits(nc, limit=1):
    """walrus CoreV2/V3 lowering rejects instructions whose sync_info carries
    more than ~1 wait condition. Hoist excess waits onto same-engine NoOps
    inserted immediately before the offending instruction (engines execute
    their stream in order, so the waits still gate it)."""
    for fn in nc.m.functions:
        for blk in fn.blocks:
            newlist = []
            for inst in blk.instructions:
                si = inst.sync_info
                if si is not None and si.on_wait and len(si.on_wait) > limit:
                    waits = list(si.on_wait)
                    extra, keep = waits[:-limit], waits[-limit:]
                    for j, w in enumerate(extra):
                        pre = mybir.InstNoOp(
                            name=f"{inst.name}_w{j}",
                            sync_info=mybir.SyncInfo(on_wait=[w], on_update=[]),
                            bass_nofuse=True,
                            engine=inst.engine,
                        )
                        nc.register_instruction(pre, overwrite=True)
                        newlist.append(pre)
                    inst.sync_info = copy.replace(si, on_wait=keep)
                newlist.append(inst)
            blk.instructions = newlist


def build_kernel(t_steps=T):
    nblk = (t_steps + 7) // 8
    xt_blocks = nblk + 8
    n_steps = t_steps + L - 1  # wavefront steps

    nc = bass.Bass(trn_type="TRN2")
    xt = nc.dram_tensor("xt", [xt_blocks * 128, BC], F16, kind="ExternalInput")
    sinit = nc.dram_tensor("sinit", [128, 2 * BC], F16, kind="ExternalInput")
    wbuf = nc.dram_tensor("wbuf", [128, W_SLOTS * 32], F16, kind="ExternalInput")
    y = nc.dram_tensor("y", [1, BC], F32, kind="ExternalOutput")

    with tile.TileContext(nc) as tc:
        with (
            tc.tile_pool(name="persist", bufs=1) as pp,
            tc.tile_pool(name="psum", bufs=1, space="PSUM") as pq,
        ):
            wt = pp.tile([128, W_SLOTS * 32], F16, tag="wt", name="wt")
            ring = [pp.tile([128, BC], F16, tag=f"ring{i}", name=f"ring{i}") for i in range(RING)]
            sa = [pp.tile([128, 2 * BC], F16, tag=f"sa{i}", name=f"sa{i}") for i in range(2)]
            sb = [pp.tile([128, BC], F16, tag=f"sb{i}", name=f"sb{i}") for i in range(2)]
            pa = [pq.tile([128, 2 * BC], F32, tag=f"pa{i}", name=f"pa{i}") for i in range(2)]
            pb = [pq.tile([128, BC], F32, tag=f"pb{i}", name=f"pb{i}") for i in range(2)]
            pfc = pq.tile([1, BC], F32, tag="pfc", name="pfc")
            yout = pp.tile([1, BC], F32, tag="yout", name="yout")

            # --- init ---
            nc.sync.dma_start(out=wt[:, :], in_=wbuf[:, :])
            for i in range(RING):
                nc.sync.dma_start(out=ring[i][:, :],
                                  in_=xt[i * 128:(i + 1) * 128, :])
            for p_ in pa:
                nc.vector.memset(p_[:, :], 0.0)
            for p_ in pb:
                nc.vector.memset(p_[:, :], 0.0)
            nc.vector.memset(pfc[:, :], 0.0)
            for s_ in sa:
                nc.sync.dma_start(out=s_[:, :], in_=sinit[:, :])
            for s_ in sb:
                nc.sync.dma_start(out=s_[:, :], in_=sinit[:, 0:BC])

            def w_ap(slot, cols):
                base = slot * 32
                return wt[:, base:base + cols]

            # (weight slot, rhs window fn, out psum fn, out base, M)
            #   rhs windows: r0 = sa[j][:, 0:128], r1 = sa[j][:, 128:256],
            #                r2 = sb[j][:, :]
            def emit_step(s):
                j = (s - 1) % 2   # buffers holding step s-1 state
                k = s % 2         # buffers for this step's outputs
                r0 = sa[j][:, 0:BC]
                r1 = sa[j][:, BC:2 * BC]
                r2 = sb[j][:, :]

                if s % 8 == 0:
                    b = s // 8 + 4
                    if b < xt_blocks:
                        nc.sync.dma_start(out=ring[b % RING][:, :],
                                          in_=xt[b * 128:(b + 1) * 128, :])

                # ---- R0/R1 matmuls -> pa[k] ----
                if s < t_steps:  # inject + L0 rec (area slot0 of R0)
                    blk = (s // 8) % RING
                    nc.tensor.matmul(pa[k][0:30, 0:BC], w_ap(W_INJ(s % 8), 30),
                                     ring[blk][:, :], start=True, stop=False,
                                     tile_position=(0, 0))
                    nc.tensor.matmul(pa[k][0:30, 0:BC], w_ap(W_L0REC, 30),
                                     r0, start=False, stop=True,
                                     tile_position=(0, 0))
                if s <= t_steps:      # L1
                    nc.tensor.matmul(pa[k][32:62, 0:BC], w_ap(W_L(1), 30),
                                     r0, start=True, stop=True,
                                     tile_position=(0, 32))
                if s <= t_steps + 1:  # L2
                    nc.tensor.matmul(pa[k][64:94, 0:BC], w_ap(W_L(2), 30),
                                     r0, start=True, stop=True,
                                     tile_position=(0, 64))
                if s <= t_steps + 2:  # L3 main (M=31: ones regen) + dup3
                    nc.tensor.matmul(pa[k][96:127, 0:BC], w_ap(W_L(3), 31),
                                     r0, start=True, stop=True,
                                     tile_position=(0, 96))
                    nc.tensor.matmul(pa[k][0:30, BC:2 * BC], w_ap(W_L(3), 30),
                                     r0, start=True, stop=True,
                                     tile_position=(0, 0))
                if s <= t_steps + 3:  # L4
                    nc.tensor.matmul(pa[k][32:62, BC:2 * BC], w_ap(W_L(4), 30),
                                     r1, start=True, stop=True,
                                     tile_position=(0, 32))
                if s <= t_steps + 4:  # L5
                    nc.tensor.matmul(pa[k][64:94, BC:2 * BC], w_ap(W_L(5), 30),
                                     r1, start=True, stop=True,
                                     tile_position=(0, 64))
                if s <= t_steps + 5:  # L6 main (ones regen) + dup6 -> pb
                    nc.tensor.matmul(pa[k][96:127, BC:2 * BC], w_ap(W_L(6), 31),
                                     r1, start=True, stop=True,
                                     tile_position=(0, 96))
                    nc.tensor.matmul(pb[k][0:30, 0:BC], w_ap(W_L(6), 30),
                                     r1, start=True, stop=True,
                                     tile_position=(0, 0))
                if s <= t_steps + 6:  # L7
                    nc.tensor.matmul(pb[k][32:62, 0:BC], w_ap(W_L(7), 30),
                                     r2, start=True, stop=True,
                                     tile_position=(0, 32))
                if s <= t_steps + 7:  # L8
                    nc.tensor.matmul(pb[k][64:94, 0:BC], w_ap(W_L(8), 30),
                                     r2, start=True, stop=True,
                                     tile_position=(0, 64))
                # L9 always runs (s <= t_steps + 8 == n_steps - 1)
                nc.tensor.matmul(pb[k][96:127, 0:BC], w_ap(W_L(9), 31),
                                 r2, start=True, stop=True,
                                 tile_position=(0, 96))

                # ---- activations ----
                nc.scalar.activation(sa[k][:, :], pa[k][:, :],
                                     mybir.ActivationFunctionType.Tanh)
                nc.scalar.activation(sb[k][:, :], pb[k][:, :],
                                     mybir.ActivationFunctionType.Tanh)

                # ---- warmup zeroing: slot h^(s+1) must be 0 before step s+1
                l = s + 1
                if 1 <= l <= 9:
                    tgt = [None,
                           (sa, 32, 62, 0), (sa, 64, 94, 0), (sa, 96, 126, 0),
                           (sa, 32, 62, 1), (sa, 64, 94, 1), (sa, 96, 126, 1),
                           (sb, 32, 62, 0), (sb, 64, 94, 0), (sb, 96, 126, 0),
                           ][l]
                    buf, p0, p1, half = tgt
                    if buf is sa:
                        nc.sync.dma_start(
                            out=sa[k][p0:p1, half * BC:(half + 1) * BC],
                            in_=sinit[0:p1 - p0, 0:BC])
                    else:
                        nc.sync.dma_start(out=sb[k][p0:p1, :],
                                          in_=sinit[0:p1 - p0, 0:BC])

            for s in range(n_steps):
                emit_step(s)

            # ---- FC on h9 of last timestep (in sb[(n_steps-1)%2] slot 3) ----
            fin = sb[(n_steps - 1) % 2][:, :]
            nc.tensor.matmul(pfc[0:1, :], w_ap(W_FC, 1), fin,
                             start=True, stop=True, tile_position=(0, 0))
            nc.vector.tensor_copy(yout[0:1, :], pfc[0:1, :])
            nc.sync.dma_start(out=y[:, :], in_=yout[0:1, :])

    _split_sync_waits(nc)
    return nc


def prep_core_inputs(x_core, W_ih0, W_ih, W_hh, b_ih, b_hh, fc_w, fc_b,
                     t_steps=T):
    """Host-side marshaling for one core. x_core: [BC, t_steps, D] fp32."""
    nblk = (t_steps + 7) // 8
    xt_blocks = nblk + 8
    # XT[g*128 + 16*(t%8)+d, b] = x[b, 8g + t%8, d]
    xt = np.zeros((xt_blocks * 128, BC), np.float16)
    xr = np.transpose(x_core, (1, 2, 0))  # [t, d, b]
    xr = xr.reshape(nblk * 8 if t_steps % 8 == 0 else t_steps, D, BC)
    tpad = nblk * 8
    if t_steps != tpad:
        xr = np.concatenate([xr, np.zeros((tpad - t_steps, D, BC), xr.dtype)], 0)
    xt[:nblk * 128, :] = xr.reshape(nblk, 8 * D, BC).reshape(nblk * 128, BC)

    wbuf = np.zeros((128, W_SLOTS * 32), np.float32)

    def put(slot, rows, col0, mat):
        # mat [m, k] -> wbuf[rows0+k, slot*32 + col0 + m]
        base = slot * 32
        wbuf[rows:rows + mat.shape[1], base + col0: base + col0 + mat.shape[0]] = mat.T

    # L0 rec: window R0, rec slot 0
    put(W_L0REC, 0, 0, W_hh[0])
    wbuf[126, W_L0REC * 32 + 0: W_L0REC * 32 + 30] = b_ih[0] + b_hh[0]
    # layers 1..9: feed slot (lp), rec slot (lp+1) in their window
    for l in range(1, 10):
        slot = W_L(l)
        jin = (l - 1) % 3   # position within window: R0 l=1,2,3 -> 0,1,2
        put(slot, 32 * jin, 0, W_ih[l - 1])
        put(slot, 32 * (jin + 1), 0, W_hh[l])
        wbuf[126, slot * 32: slot * 32 + 30] = b_ih[l] + b_hh[l]
        if jin == 2:  # slot-3 output: ones-regen column 30
            wbuf[126, slot * 32 + 30] = 30.0
    # inject variants
    for v in range(8):
        put(W_INJ(v), 16 * v, 0, W_ih0)
    # FC
    wbuf[96:126, W_FC * 32] = fc_w[0]
    wbuf[126, W_FC * 32] = fc_b[0]

    sinit = np.zeros((128, 2 * BC), np.float16)
    sinit[126, :] = 1.0
    return {"xt": xt, "wbuf": wbuf.astype(np.float16), "sinit": sinit}


_CACHE = {}


def run(x, W_ih0, W_ih, W_hh, b_ih, b_hh, fc_w, fc_b, t_steps=T):
    x = np.asarray(x, np.float32)
    args = [np.asarray(a, np.float32) for a in
            (W_ih0, W_ih, W_hh, b_ih, b_hh, fc_w, fc_b)]
    key = t_steps
    if key not in _CACHE:
        _CACHE[key] = build_kernel(t_steps)
    nc = _CACHE[key]
    in_maps = [prep_core_inputs(x[c * BC:(c + 1) * BC], *args, t_steps=t_steps)
               for c in range(N_CORES)]
    res = bass_utils.run_bass_kernel_spmd(nc, in_maps,
                                          core_ids=list(range(N_CORES)))
    out = np.concatenate([res.results[c]["y"].reshape(BC, 1)
                          for c in range(N_CORES)], axis=0)
    return out, res


def kernel(x, W_ih0, W_ih, W_hh, b_ih, b_hh, fc_w, fc_b):
    out, _ = run(x, W_ih0, W_ih, W_hh, b_ih, b_hh, fc_w, fc_b)
    return out


if __name__ == "__main__":
    # quick small-T shakedown vs numpy
    t_small = 32
    rng = np.random.default_rng(0)
    s = 1.0 / np.sqrt(H)
    x = rng.standard_normal((B, t_small, D)).astype(np.float32)
    W_ih0 = (rng.standard_normal((H, D)) * s).astype(np.float32)
    W_ih = (rng.standard_normal((L - 1, H, H)) * s).astype(np.float32)
    W_hh = (rng.standard_normal((L, H, H)) * s).astype(np.float32)
    b_ih = (rng.standard_normal((L, H)) * s).astype(np.float32)
    b_hh = (rng.standard_normal((L, H)) * s).astype(np.float32)
    fc_w = (rng.standard_normal((1, H)) * s).astype(np.float32)
    fc_b = (rng.standard_normal((1,)) * s).astype(np.float32)

    def ref_np(x):
        out = x
        for l in range(L):
            Wi = W_ih0 if l == 0 else W_ih[l - 1]
            xw = np.einsum("btd,hd->bth", out, Wi) + (b_ih[l] + b_hh[l])
            h = np.zeros((x.shape[0], H), np.float32)
            ys = np.empty((x.shape[0], xw.shape[1], H), np.float32)
            for t in range(xw.shape[1]):
                h = np.tanh(xw[:, t] + h @ W_hh[l].T)
                ys[:, t] = h
            out = ys
        return out[:, -1, :] @ fc_w.T + fc_b

    want = ref_np(x)
    got, _ = run(x, W_ih0, W_ih, W_hh, b_ih, b_hh, fc_w, fc_b, t_steps=t_small)
    err = np.abs(got - want).max() / (np.abs(want).max() + 1e-9)
    print("small-T rel err:", err)
    print("sample got:", got[:4, 0], "want:", want[:4, 0])
